# revision 1
# baseline (speedup 1.0000x reference)
"""GCN forward on 8 Trainium2 NeuronCores.

Reference computation:
  h1 = relu(GCNConv(x, edge_index; w_conv, b_conv))      [20000, 32]
  h3 = relu(h1.flatten() @ w_fc1.T + b_fc1)              [128]
  out = relu(h3 @ w_fc2.T + b_fc2)                       [1, 20000]

Strategy (all 8 cores, SPMD, one NEFF):
  - GCNConv aggregation as a DENSE matmul: A_hat = D^-1/2 (A+I) D^-1/2 where
    (A+I) holds small integer edge counts, exactly representable in bf16.
    dinv[src] is folded into the H'' rows, dinv[dst] applied post-matmul.
    Each core owns a 2500-node dst slice: psum[dst_tile] += Apack_tile.T @ H''.
  - H'' = dinv * (x @ w_conv) computed sharded (each core its 2500 src rows),
    then AllGather (bf16, 160KB/core).
  - fc1 column-sharded: core i dots its 80000 flat entries against its B
    slice, AllReduce of the [128] partials.
  - fc2 row-sharded: core i computes out[2500i:2500(i+1)].
"""
import numpy as np
import ml_dtypes

N = 20000
IN_FEAT = 128
CF = 32            # conv out feats
FC1 = 128
NC_ = 8            # cores
NS = N // NC_      # 2500 nodes per core
DT = 20            # dst tiles per core (last partial: 68 rows)
KT = (N + 127) // 128  # 157 src tiles
KT_A = 79          # first  src-tile chunk
KT_B = KT - KT_A   # second src-tile chunk (78)

_BF16 = ml_dtypes.bfloat16


def _host_prep(x, edge_index, w_conv, b_conv, w_fc1, b_fc1, w_fc2, b_fc2):
    src = edge_index[0].astype(np.int64)
    dst = edge_index[1].astype(np.int64)
    deg = np.bincount(dst, minlength=N).astype(np.float32) + 1.0
    dinv = (1.0 / np.sqrt(deg)).astype(np.float32)

    x = np.asarray(x, np.float32)
    w_conv = np.asarray(w_conv, np.float32)
    b_conv = np.asarray(b_conv, np.float32)
    w_fc1 = np.asarray(w_fc1, np.float32)
    b_fc1 = np.asarray(b_fc1, np.float32)
    w_fc2 = np.asarray(w_fc2, np.float32)
    b_fc2 = np.asarray(b_fc2, np.float32)

    lut = np.arange(256).astype(_BF16)  # exact small-int -> bf16
    bconvb = np.ascontiguousarray(np.broadcast_to(b_conv[None, :], (128, CF)))
    bfc1c = np.ascontiguousarray(b_fc1.reshape(128, 1))

    in_maps = []
    for c in range(NC_):
        base = c * NS
        # xt: [128 feat, 2560 nodes] zero-padded
        xt = np.zeros((IN_FEAT, DT * 128), np.float32)
        xt[:, :NS] = x[base:base + NS].T
        # dinv tile [128, 20], zero-padded
        dv = np.zeros(DT * 128, np.float32)
        dv[:NS] = dinv[base:base + NS]
        dv = np.ascontiguousarray(dv.reshape(DT, 128).T)
        # A_pack[d, p, k*128+j] = count(src=128k+p -> dst=base+128d+j) + selfloop
        cnt = np.zeros((DT, 128, KT * 128), np.uint8)
        m = (dst >= base) & (dst < base + NS)
        s, dl = src[m], dst[m] - base
        np.add.at(cnt, (dl // 128, s % 128, (s // 128) * 128 + dl % 128), 1)
        v = np.arange(base, base + NS)
        np.add.at(cnt, ((v - base) // 128, v % 128, (v // 128) * 128 + (v - base) % 128), 1)
        apack = lut[cnt]
        del cnt
        # B_pack[k, n, c, o] = w_fc1[o, 80000*i + 32*(128k+n) + c], zero-padded
        w1 = w_fc1[:, base * CF:(base + NS) * CF]  # [128, 80000]
        bp = np.zeros((DT, 128, CF, FC1), _BF16)
        bp[:19] = w1[:, :19 * 128 * CF].reshape(FC1, 19, 128, CF).transpose(1, 2, 3, 0).astype(_BF16)
        bp[19, :NS - 19 * 128] = w1[:, 19 * 128 * CF:].reshape(FC1, NS - 19 * 128, CF).transpose(1, 2, 0).astype(_BF16)
        in_maps.append({
            "xt": xt,
            "wconv": np.ascontiguousarray(w_conv),
            "dinv": dv,
            "bconvb": bconvb,
            "apack": apack.reshape(DT, 128, KT * 128),
            "bpack": np.ascontiguousarray(bp.reshape(DT, 128, CF * FC1)),
            "bfc1": bfc1c,
            "w2t": np.ascontiguousarray(w_fc2[base:base + NS].T),
            "bfc2": np.ascontiguousarray(b_fc2[base:base + NS].reshape(1, NS)),
        })
    return in_maps


def _build_bass():
    import concourse.bass as bass
    import concourse.mybir as mybir
    import concourse.tile as tile
    from concourse import bacc

    F32, BF16 = mybir.dt.float32, mybir.dt.bfloat16
    nc = bacc.Bacc("TRN2", target_bir_lowering=False, debug=False, num_devices=NC_)

    xt = nc.dram_tensor("xt", [IN_FEAT, DT * 128], F32, kind="ExternalInput")
    wconv = nc.dram_tensor("wconv", [IN_FEAT, CF], F32, kind="ExternalInput")
    dinv = nc.dram_tensor("dinv", [128, DT], F32, kind="ExternalInput")
    bconvb = nc.dram_tensor("bconvb", [128, CF], F32, kind="ExternalInput")
    apack = nc.dram_tensor("apack", [DT, 128, KT * 128], BF16, kind="ExternalInput")
    bpack = nc.dram_tensor("bpack", [DT, 128, CF * FC1], BF16, kind="ExternalInput")
    bfc1 = nc.dram_tensor("bfc1", [FC1, 1], F32, kind="ExternalInput")
    w2t = nc.dram_tensor("w2t", [FC1, NS], F32, kind="ExternalInput")
    bfc2 = nc.dram_tensor("bfc2", [1, NS], F32, kind="ExternalInput")
    out = nc.dram_tensor("out", [1, NS], F32, kind="ExternalOutput")

    hq_in = nc.dram_tensor("hq_in", [NS, CF], BF16)
    hq_out = nc.dram_tensor("hq_out", [N, CF], BF16, addr_space="Shared")
    p_in = nc.dram_tensor("p_in", [1, FC1], F32)
    p_out = nc.dram_tensor("p_out", [1, FC1], F32, addr_space="Shared")

    with tile.TileContext(nc) as tc:
        with tc.tile_pool(name="const", bufs=1) as cp, \
             tc.tile_pool(name="work", bufs=3) as wp, \
             tc.tile_pool(name="ps", bufs=2, space="PSUM") as pp, \
             tc.tile_pool(name="ps1", bufs=1, space="PSUM") as pp1:

            xt_sb = cp.tile([IN_FEAT, DT * 128], F32, tag="xt")
            nc.sync.dma_start(out=xt_sb[:], in_=xt[:])
            wconv_sb = cp.tile([IN_FEAT, CF], F32, tag="wconv")
            nc.sync.dma_start(out=wconv_sb[:], in_=wconv[:])
            dinv_sb = cp.tile([128, DT], F32, tag="dinv")
            nc.sync.dma_start(out=dinv_sb[:], in_=dinv[:])
            bconvb_sb = cp.tile([128, CF], F32, tag="bconvb")
            nc.sync.dma_start(out=bconvb_sb[:], in_=bconvb[:])
            bfc1_sb = cp.tile([FC1, 1], F32, tag="bfc1")
            nc.sync.dma_start(out=bfc1_sb[:], in_=bfc1[:])
            w2t_sb = cp.tile([FC1, NS], F32, tag="w2t")
            nc.sync.dma_start(out=w2t_sb[:], in_=w2t[:])
            bfc2_sb = cp.tile([1, NS], F32, tag="bfc2")
            nc.sync.dma_start(out=bfc2_sb[:], in_=bfc2[:])

            # ---- S1: H'' = dinv * (x @ w_conv) for own src slice (bf16) ----
            hq_sb = cp.tile([128, DT * CF], BF16, tag="hq")
            for k in range(DT):
                ps = pp.tile([128, CF], F32, space="PSUM", tag="ps")
                nc.tensor.matmul(out=ps[:], lhsT=xt_sb[:, k * 128:(k + 1) * 128],
                                 rhs=wconv_sb[:], start=True, stop=True)
                nc.vector.tensor_tensor(out=hq_sb[:, k * CF:(k + 1) * CF], in0=ps[:],
                                        in1=dinv_sb[:, k:k + 1].to_broadcast([128, CF]),
                                        op=mybir.AluOpType.mult)
            # store rows 0:2432 then tail 2432:2500
            nc.sync.dma_start(out=hq_in[:19 * 128].rearrange("(k p) f -> p k f", p=128),
                              in_=hq_sb[:, :19 * CF].rearrange("p (k f) -> p k f", f=CF))
            nc.sync.dma_start(out=hq_in[19 * 128:NS], in_=hq_sb[:NS - 19 * 128, 19 * CF:20 * CF])

            # ---- S2: AllGather H'' ----
            nc.gpsimd.collective_compute(
                "AllGather", mybir.AluOpType.bypass,
                replica_groups=[list(range(NC_))],
                ins=[hq_in[:]], outs=[hq_out[:]])

            # ---- S3: load full H'' into SBUF [128, 157*32] ----
            hp = cp.tile([128, KT * CF], BF16, tag="hp")
            nc.vector.memset(hp[:], 0.0)
            nc.sync.dma_start(out=hp[:, :156 * CF].rearrange("p (k f) -> p k f", f=CF),
                              in_=hq_out[:156 * 128].rearrange("(k p) f -> p k f", p=128))
            nc.sync.dma_start(out=hp[:N - 156 * 128, 156 * CF:], in_=hq_out[156 * 128:])

            # ---- S4: aggregation, one dst tile at a time ----
            h1_sb = cp.tile([128, DT * CF], BF16, tag="h1")
            for d in range(DT):
                a1 = wp.tile([128, KT_A * 128], BF16, tag="apk")
                nc.sync.dma_start(out=a1[:], in_=apack[d, :, :KT_A * 128])
                a2 = wp.tile([128, KT_B * 128], BF16, tag="apk")
                nc.sync.dma_start(out=a2[:], in_=apack[d, :, KT_A * 128:])
                psd = pp.tile([128, CF], F32, space="PSUM", tag="ps")
                for k in range(KT):
                    at = a1 if k < KT_A else a2
                    kk = k if k < KT_A else k - KT_A
                    nc.tensor.matmul(out=psd[:], lhsT=at[:, kk * 128:(kk + 1) * 128],
                                     rhs=hp[:, k * CF:(k + 1) * CF],
                                     start=(k == 0), stop=(k == KT - 1))
                t = wp.tile([128, CF], F32, tag="ep")
                nc.vector.tensor_tensor(out=t[:], in0=psd[:],
                                        in1=dinv_sb[:, d:d + 1].to_broadcast([128, CF]),
                                        op=mybir.AluOpType.mult)
                nc.vector.tensor_tensor(out=t[:], in0=t[:], in1=bconvb_sb[:],
                                        op=mybir.AluOpType.add)
                nc.scalar.activation(out=h1_sb[:, d * CF:(d + 1) * CF], in_=t[:],
                                     func=mybir.ActivationFunctionType.Relu)

            # ---- S5: fc1 partial: ps1[1, 128] += h1_col.T @ B_tile ----
            ps1 = pp1.tile([1, FC1], F32, space="PSUM", tag="ps1")
            for k in range(DT):
                bsb = wp.tile([128, CF * FC1], BF16, tag="bpk")
                nc.sync.dma_start(out=bsb[:], in_=bpack[k])
                for c in range(CF):
                    nc.tensor.matmul(out=ps1[:], lhsT=h1_sb[:, k * CF + c:k * CF + c + 1],
                                     rhs=bsb[:, c * FC1:(c + 1) * FC1],
                                     start=(k == 0 and c == 0), stop=(k == DT - 1 and c == CF - 1))
            p_sb = cp.tile([1, FC1], F32, tag="p_sb")
            nc.vector.tensor_copy(out=p_sb[:], in_=ps1[:])
            nc.gpsimd.dma_start(out=p_in[:], in_=p_sb[:])

            # ---- S6: AllReduce partials, h3 = relu(sum + b_fc1) ----
            nc.gpsimd.collective_compute(
                "AllReduce", mybir.AluOpType.add,
                replica_groups=[list(range(NC_))],
                ins=[p_in[:]], outs=[p_out[:]])
            h3 = cp.tile([FC1, 1], F32, tag="h3")
            nc.sync.dma_start(out=h3[:], in_=p_out[:].rearrange("a k -> k a"))
            nc.vector.tensor_tensor(out=h3[:], in0=h3[:], in1=bfc1_sb[:],
                                    op=mybir.AluOpType.add)
            nc.scalar.activation(out=h3[:], in_=h3[:],
                                 func=mybir.ActivationFunctionType.Relu)

            # ---- S7: fc2 slice: out = relu(h3.T @ w2t + bfc2) ----
            o_sb = cp.tile([1, NS], F32, tag="o_sb")
            for j in range(5):
                ps2 = pp.tile([1, 500], F32, space="PSUM", tag="ps2")
                nc.tensor.matmul(out=ps2[:], lhsT=h3[:],
                                 rhs=w2t_sb[:, j * 500:(j + 1) * 500], start=True, stop=True)
                nc.vector.tensor_tensor(out=o_sb[:, j * 500:(j + 1) * 500], in0=ps2[:],
                                        in1=bfc2_sb[:, j * 500:(j + 1) * 500],
                                        op=mybir.AluOpType.add)
            nc.scalar.activation(out=o_sb[:], in_=o_sb[:],
                                 func=mybir.ActivationFunctionType.Relu)
            nc.sync.dma_start(out=out[:], in_=o_sb[:])

    nc.finalize()
    return nc


_CACHED = {}


def kernel(**inputs) -> np.ndarray:
    from concourse.bass_utils import run_bass_kernel_spmd

    in_maps = _host_prep(**inputs)
    if "nc" not in _CACHED:
        _CACHED["nc"] = _build_bass()
    nc = _CACHED["nc"]
    res = run_bass_kernel_spmd(nc, in_maps, core_ids=list(range(NC_)))
    return np.concatenate([res.results[c]["out"] for c in range(NC_)], axis=1)


# revision 3
# speedup vs baseline: 28934.0128x; 28934.0128x over previous
"""GCN forward on 8 Trainium2 NeuronCores.

Reference computation:
  h1 = relu(GCNConv(x, edge_index; w_conv, b_conv))      [20000, 32]
  h3 = relu(h1.flatten() @ w_fc1.T + b_fc1)              [128]
  out = relu(h3 @ w_fc2.T + b_fc2)                       [1, 20000]

Strategy (all 8 cores, SPMD, one NEFF):
  - GCNConv aggregation as a DENSE matmul: A_hat = D^-1/2 (A+I) D^-1/2 where
    (A+I) holds small integer edge counts, exactly representable in bf16.
    dinv[src] is folded into the H'' rows, dinv[dst] applied post-matmul.
    Each core owns a 2500-node dst slice: psum[dst_tile] += Apack_tile.T @ H''.
  - H'' = dinv * (x @ w_conv) computed sharded (each core its 2500 src rows),
    then AllGather (bf16, 160KB/core).
  - fc1 column-sharded: core i dots its 80000 flat entries against its B
    slice, AllReduce of the [128] partials.
  - fc2 row-sharded: core i computes out[2500i:2500(i+1)].
"""
import numpy as np
import ml_dtypes

N = 20000
IN_FEAT = 128
CF = 32            # conv out feats
FC1 = 128
NC_ = 8            # cores
NS = N // NC_      # 2500 nodes per core
DT = 20            # dst tiles per core (last partial: 68 rows)
KT = (N + 127) // 128  # 157 src tiles
KT_A = 79          # first  src-tile chunk
KT_B = KT - KT_A   # second src-tile chunk (78)

_BF16 = ml_dtypes.bfloat16
_F8 = ml_dtypes.float8_e4m3


def _host_prep(x, edge_index, w_conv, b_conv, w_fc1, b_fc1, w_fc2, b_fc2):
    src = edge_index[0].astype(np.int64)
    dst = edge_index[1].astype(np.int64)
    deg = np.bincount(dst, minlength=N).astype(np.float32) + 1.0
    dinv = (1.0 / np.sqrt(deg)).astype(np.float32)

    x = np.asarray(x, np.float32)
    w_conv = np.asarray(w_conv, np.float32)
    b_conv = np.asarray(b_conv, np.float32)
    w_fc1 = np.asarray(w_fc1, np.float32)
    b_fc1 = np.asarray(b_fc1, np.float32)
    w_fc2 = np.asarray(w_fc2, np.float32)
    b_fc2 = np.asarray(b_fc2, np.float32)

    lut = np.arange(16).astype(_F8)  # exact small-int -> fp8e4m3
    bconvb = np.ascontiguousarray(np.broadcast_to(b_conv[None, :], (128, CF)))
    bfc1c = np.ascontiguousarray(b_fc1.reshape(128, 1))

    in_maps = []
    for c in range(NC_):
        base = c * NS
        # xt: [128 feat, 2560 nodes] zero-padded
        xt = np.zeros((IN_FEAT, DT * 128), np.float32)
        xt[:, :NS] = x[base:base + NS].T
        # dinv tile [128, 20], zero-padded
        dv = np.zeros(DT * 128, np.float32)
        dv[:NS] = dinv[base:base + NS]
        dv = np.ascontiguousarray(dv.reshape(DT, 128).T)
        # A_pack[d, p, k*128+j] = count(src=128k+p -> dst=base+128d+j) + selfloop
        cnt = np.zeros((DT, 128, KT * 128), np.uint8)
        m = (dst >= base) & (dst < base + NS)
        s, dl = src[m], dst[m] - base
        np.add.at(cnt, (dl // 128, s % 128, (s // 128) * 128 + dl % 128), 1)
        v = np.arange(base, base + NS)
        np.add.at(cnt, ((v - base) // 128, v % 128, (v // 128) * 128 + (v - base) % 128), 1)
        assert cnt.max() < 16, cnt.max()
        apack = lut[cnt]
        del cnt
        # B_pack[k, n, c, o] = w_fc1[o, 80000*i + 32*(128k+n) + c], zero-padded
        w1 = w_fc1[:, base * CF:(base + NS) * CF]  # [128, 80000]
        bp = np.zeros((DT, 128, CF, FC1), _BF16)
        bp[:19] = w1[:, :19 * 128 * CF].reshape(FC1, 19, 128, CF).transpose(1, 2, 3, 0).astype(_BF16)
        bp[19, :NS - 19 * 128] = w1[:, 19 * 128 * CF:].reshape(FC1, NS - 19 * 128, CF).transpose(1, 2, 0).astype(_BF16)
        in_maps.append({
            "xt": xt,
            "wconv": np.ascontiguousarray(w_conv),
            "dinv": dv,
            "bconvb": bconvb,
            "apack": apack.reshape(DT, 128, KT * 128),
            "bpack": np.ascontiguousarray(bp.reshape(DT, 128, CF * FC1)),
            "bfc1": bfc1c,
            "w2t": np.ascontiguousarray(w_fc2[base:base + NS].T),
            "bfc2": np.ascontiguousarray(b_fc2[base:base + NS].reshape(1, NS)),
        })
    return in_maps


def _build_bass(timing_reps=None):
    import concourse.bass as bass
    import concourse.mybir as mybir
    import concourse.tile as tile
    from concourse import bacc

    F32, BF16, F8 = mybir.dt.float32, mybir.dt.bfloat16, mybir.dt.float8e4
    import contextlib
    nc = bacc.Bacc("TRN2", target_bir_lowering=False, debug=False,
                   num_devices=1 if timing_reps else NC_)

    xt = nc.dram_tensor("xt", [IN_FEAT, DT * 128], F32, kind="ExternalInput")
    wconv = nc.dram_tensor("wconv", [IN_FEAT, CF], F32, kind="ExternalInput")
    dinv = nc.dram_tensor("dinv", [128, DT], F32, kind="ExternalInput")
    bconvb = nc.dram_tensor("bconvb", [128, CF], F32, kind="ExternalInput")
    apack = nc.dram_tensor("apack", [DT, 128, KT * 128], F8, kind="ExternalInput")
    bpack = nc.dram_tensor("bpack", [DT, 128, CF * FC1], BF16, kind="ExternalInput")
    bfc1 = nc.dram_tensor("bfc1", [FC1, 1], F32, kind="ExternalInput")
    w2t = nc.dram_tensor("w2t", [FC1, NS], F32, kind="ExternalInput")
    bfc2 = nc.dram_tensor("bfc2", [1, NS], F32, kind="ExternalInput")
    out = nc.dram_tensor("out", [1, NS], F32, kind="ExternalOutput")

    hq_in = nc.dram_tensor("hq_in", [NS, CF], BF16)
    hq_out = nc.dram_tensor("hq_out", [N, CF], BF16, addr_space="Shared")
    p_in = nc.dram_tensor("p_in", [1, FC1], F32)
    p_out = nc.dram_tensor("p_out", [1, FC1], F32, addr_space="Shared")

    with tile.TileContext(nc) as tc:
        with tc.tile_pool(name="const", bufs=1) as cp, \
             tc.tile_pool(name="work", bufs=3) as wp, \
             tc.tile_pool(name="ps", bufs=2, space="PSUM") as pp, \
             tc.tile_pool(name="ps1", bufs=1, space="PSUM") as pp1:

            xt_sb = cp.tile([IN_FEAT, DT * 128], F32, tag="xt")
            nc.sync.dma_start(out=xt_sb[:], in_=xt[:])
            wconv_sb = cp.tile([IN_FEAT, CF], F32, tag="wconv")
            nc.sync.dma_start(out=wconv_sb[:], in_=wconv[:])
            dinv_sb = cp.tile([128, DT], F32, tag="dinv")
            nc.sync.dma_start(out=dinv_sb[:], in_=dinv[:])
            bconvb_sb = cp.tile([128, CF], F32, tag="bconvb")
            nc.sync.dma_start(out=bconvb_sb[:], in_=bconvb[:])
            bfc1_sb = cp.tile([FC1, 1], F32, tag="bfc1")
            nc.sync.dma_start(out=bfc1_sb[:], in_=bfc1[:])
            w2t_sb = cp.tile([FC1, NS], F32, tag="w2t")
            nc.sync.dma_start(out=w2t_sb[:], in_=w2t[:])
            bfc2_sb = cp.tile([1, NS], F32, tag="bfc2")
            nc.sync.dma_start(out=bfc2_sb[:], in_=bfc2[:])

            loop_cm = tc.For_i(0, timing_reps, 1) if timing_reps else contextlib.nullcontext()
            loop_cm.__enter__()

            # ---- S1: H'' = dinv * (x @ w_conv) for own src slice (bf16) ----
            hq_sb = cp.tile([128, DT * CF], BF16, tag="hq")
            for k in range(DT):
                ps = pp.tile([128, CF], F32, space="PSUM", tag="ps")
                nc.tensor.matmul(out=ps[:], lhsT=xt_sb[:, k * 128:(k + 1) * 128],
                                 rhs=wconv_sb[:], start=True, stop=True)
                nc.vector.tensor_tensor(out=hq_sb[:, k * CF:(k + 1) * CF], in0=ps[:],
                                        in1=dinv_sb[:, k:k + 1].to_broadcast([128, CF]),
                                        op=mybir.AluOpType.mult)
            # store rows 0:2432 then tail 2432:2500
            nc.sync.dma_start(out=hq_in[:19 * 128].rearrange("(k p) f -> p k f", p=128),
                              in_=hq_sb[:, :19 * CF].rearrange("p (k f) -> p k f", f=CF))
            nc.sync.dma_start(out=hq_in[19 * 128:NS], in_=hq_sb[:NS - 19 * 128, 19 * CF:20 * CF])

            # ---- S2: AllGather H'' ----
            if timing_reps:
                nc.sync.dma_start(out=hq_out[:NS], in_=hq_in[:])
            else:
                nc.gpsimd.collective_compute(
                    "AllGather", mybir.AluOpType.bypass,
                    replica_groups=[list(range(NC_))],
                    ins=[hq_in[:]], outs=[hq_out[:]])

            # ---- S3: load full H'' into SBUF [128, 157*32] ----
            hp = cp.tile([128, KT * CF], BF16, tag="hp")
            nc.vector.memset(hp[:], 0.0)
            nc.sync.dma_start(out=hp[:, :156 * CF].rearrange("p (k f) -> p k f", f=CF),
                              in_=hq_out[:156 * 128].rearrange("(k p) f -> p k f", p=128))
            nc.sync.dma_start(out=hp[:N - 156 * 128, 156 * CF:], in_=hq_out[156 * 128:])

            # ---- S4: aggregation, one dst tile at a time ----
            h1_sb = cp.tile([128, DT * CF], BF16, tag="h1")
            for d in range(DT):
                a1 = wp.tile([128, KT_A * 128], F8, tag="apk")
                nc.sync.dma_start(out=a1[:], in_=apack[d, :, :KT_A * 128])
                a2 = wp.tile([128, KT_B * 128], F8, tag="apk")
                nc.sync.dma_start(out=a2[:], in_=apack[d, :, KT_A * 128:])
                psd = pp.tile([128, CF], F32, space="PSUM", tag="ps")
                for k in range(KT):
                    at = a1 if k < KT_A else a2
                    kk = k if k < KT_A else k - KT_A
                    nc.tensor.matmul(out=psd[:], lhsT=at[:, kk * 128:(kk + 1) * 128],
                                     rhs=hp[:, k * CF:(k + 1) * CF],
                                     start=(k == 0), stop=(k == KT - 1))
                t = wp.tile([128, CF], F32, tag="ep")
                nc.vector.tensor_tensor(out=t[:], in0=psd[:],
                                        in1=dinv_sb[:, d:d + 1].to_broadcast([128, CF]),
                                        op=mybir.AluOpType.mult)
                nc.vector.tensor_tensor(out=t[:], in0=t[:], in1=bconvb_sb[:],
                                        op=mybir.AluOpType.add)
                nc.scalar.activation(out=h1_sb[:, d * CF:(d + 1) * CF], in_=t[:],
                                     func=mybir.ActivationFunctionType.Relu)

            # ---- S5: fc1 partial: ps1[1, 128] += h1_col.T @ B_tile ----
            ps1 = pp1.tile([1, FC1], F32, space="PSUM", tag="ps1")
            for k in range(DT):
                bsb = wp.tile([128, CF * FC1], BF16, tag="bpk")
                nc.sync.dma_start(out=bsb[:], in_=bpack[k])
                for c in range(CF):
                    nc.tensor.matmul(out=ps1[:], lhsT=h1_sb[:, k * CF + c:k * CF + c + 1],
                                     rhs=bsb[:, c * FC1:(c + 1) * FC1],
                                     start=(k == 0 and c == 0), stop=(k == DT - 1 and c == CF - 1))
            p_sb = cp.tile([1, FC1], F32, tag="p_sb")
            nc.vector.tensor_copy(out=p_sb[:], in_=ps1[:])
            nc.gpsimd.dma_start(out=p_in[:], in_=p_sb[:])

            # ---- S6: AllReduce partials, h3 = relu(sum + b_fc1) ----
            if timing_reps:
                nc.sync.dma_start(out=p_out[:], in_=p_in[:])
            else:
                nc.gpsimd.collective_compute(
                    "AllReduce", mybir.AluOpType.add,
                    replica_groups=[list(range(NC_))],
                    ins=[p_in[:]], outs=[p_out[:]])
            h3 = cp.tile([FC1, 1], F32, tag="h3")
            nc.sync.dma_start(out=h3[:], in_=p_out[:].rearrange("a k -> k a"))
            nc.vector.tensor_tensor(out=h3[:], in0=h3[:], in1=bfc1_sb[:],
                                    op=mybir.AluOpType.add)
            nc.scalar.activation(out=h3[:], in_=h3[:],
                                 func=mybir.ActivationFunctionType.Relu)

            # ---- S7: fc2 slice: out = relu(h3.T @ w2t + bfc2) ----
            o_sb = cp.tile([1, NS], F32, tag="o_sb")
            for j in range(5):
                ps2 = pp.tile([1, 500], F32, space="PSUM", tag="ps2")
                nc.tensor.matmul(out=ps2[:], lhsT=h3[:],
                                 rhs=w2t_sb[:, j * 500:(j + 1) * 500], start=True, stop=True)
                nc.vector.tensor_tensor(out=o_sb[:, j * 500:(j + 1) * 500], in0=ps2[:],
                                        in1=bfc2_sb[:, j * 500:(j + 1) * 500],
                                        op=mybir.AluOpType.add)
            nc.scalar.activation(out=o_sb[:], in_=o_sb[:],
                                 func=mybir.ActivationFunctionType.Relu)
            nc.sync.dma_start(out=out[:], in_=o_sb[:])
            loop_cm.__exit__(None, None, None) if timing_reps else None

    nc.finalize()
    return nc


_CACHED = {}


def kernel(**inputs) -> np.ndarray:
    from concourse.bass_utils import run_bass_kernel_spmd

    in_maps = _host_prep(**inputs)
    if "nc" not in _CACHED:
        _CACHED["nc"] = _build_bass()
    nc = _CACHED["nc"]
    res = run_bass_kernel_spmd(nc, in_maps, core_ids=list(range(NC_)))
    return np.concatenate([res.results[c]["out"] for c in range(NC_)], axis=1)
